# revision 7
# baseline (speedup 1.0000x reference)
"""Trainium2 Bass kernel for a dense transformer decoder layer (no-residual variant).

Shapes (hardcoded): x [2, 2048, 768], H=12 heads, head_dim=64, FFN dim 3072, fp32 I/O.

Sharding: 8 cores; core c handles batch b=c//4, token rows [512*(c%4), 512*(c%4+1)).
Each core computes q/k/v for its own 512 rows, one AllGather per 4-core batch group
shares k (feature-major) and v (row-major, with a per-head ones column that makes the
attn@v matmul also produce the softmax denominator). Everything else is local.

Compute dtype: bf16 matmul operands, fp32 PSUM accumulation and LN/softmax math.
"""

import sys

for p in ("/opt/trn_rl_repo",):
    if p not in sys.path:
        sys.path.insert(0, p)

import numpy as np
import ml_dtypes

import concourse.bass as bass
import concourse.mybir as mybir
import concourse.tile as tile
from concourse import bacc, bass_utils
from concourse.alu_op_type import AluOpType

BF = mybir.dt.bfloat16
F32 = mybir.dt.float32
AX = mybir.AxisListType.X
ACT = mybir.ActivationFunctionType

B, T, D, F, H, HD = 2, 2048, 768, 3072, 12, 64
N_CORES = 8
ROWS = 512          # token rows per core
KT_N = D // 128     # 6 k-tiles over D
PAIRS = H // 2      # 6 head pairs
JT = T // 128       # 16 j-tiles over full sequence
VW = H * (HD + 1)   # 780: v tile width, 65 per head (64 + ones column)
EPS = 1e-5

KSZ = D * ROWS       # kT chunk elements in AG buffer
VSZ = ROWS * VW      # v chunk elements
LOCAL = KSZ + VSZ    # per-rank AG payload


def _body(nc, tc, io):
    xT, wqkv, wout, wff1, wff2 = io["xT"], io["w_qkv"], io["w_out"], io["w_ff1"], io["w_ff2"]
    bqkv, bout, bff1, bff2 = io["b_qkv"], io["b_out"], io["b_ff1"], io["b_ff2"]
    g1, bt1, g2, bt2 = io["g1"], io["bt1"], io["g2"], io["bt2"]
    out = io["out"]

    wqkv_r = wqkv.rearrange("(g p) c -> p g c", p=128)   # [128, 6, 2304]
    wout_r = wout.rearrange("(g p) c -> p g c", p=128)   # [128, 6, 768]
    wff1_r = wff1.rearrange("(g p) c -> p g c", p=128)   # [128, 6, 3072]

    with (
        tc.tile_pool(name="consts", bufs=1) as cpool,
        tc.tile_pool(name="oT", bufs=PAIRS) as oT_pool,
        tc.tile_pool(name="dram", bufs=1, space="DRAM") as dram,
    ):
        # --- constant / bias tiles ---
        bqk_t = cpool.tile([128, 12], F32, tag="bqk")     # per-partition q/k bias
        nc.sync.dma_start(bqk_t[:], bqkv[0:1536].rearrange("(j p) -> p j", p=128))
        bout_t = cpool.tile([128, 6], F32, tag="bout")
        nc.sync.dma_start(bout_t[:], bout.rearrange("(j p) -> p j", p=128))
        bff1_t = cpool.tile([128, 24], F32, tag="bff1")
        nc.sync.dma_start(bff1_t[:], bff1.rearrange("(j p) -> p j", p=128))
        g1_t = cpool.tile([128, 6], F32, tag="g1")
        nc.sync.dma_start(g1_t[:], g1.rearrange("(j p) -> p j", p=128))
        bt1_t = cpool.tile([128, 6], F32, tag="bt1")
        nc.sync.dma_start(bt1_t[:], bt1.rearrange("(j p) -> p j", p=128))

        # free-axis vectors -> broadcast to all 128 partitions
        row = cpool.tile([1, 768 * 3], F32, tag="rowvec")
        nc.sync.dma_start(row[:, 0:768], bff2[None, :])
        nc.sync.dma_start(row[:, 768:1536], g2[None, :])
        nc.sync.dma_start(row[:, 1536:2304], bt2[None, :])
        bv_row = cpool.tile([1, 768], F32, tag="bvrow")
        nc.sync.dma_start(bv_row[:], bqkv[None, 1536:2304])

        bff2_b = cpool.tile([128, 768], F32, tag="bff2b")
        g2_b = cpool.tile([128, 768], F32, tag="g2b")
        bt2_b = cpool.tile([128, 768], F32, tag="bt2b")
        bv_b = cpool.tile([128, 768], F32, tag="bvb")
        nc.gpsimd.partition_broadcast(bff2_b[:], row[:, 0:768])
        nc.gpsimd.partition_broadcast(g2_b[:], row[:, 768:1536])
        nc.gpsimd.partition_broadcast(bt2_b[:], row[:, 1536:2304])
        nc.gpsimd.partition_broadcast(bv_b[:], bv_row[:])

        ones_bf = cpool.tile([128, 1], BF, tag="ones")
        nc.gpsimd.memset(ones_bf[:], 1.0)
        eps1 = cpool.tile([1, 1], F32, tag="eps1")
        nc.gpsimd.memset(eps1[:], EPS)
        eps128 = cpool.tile([128, 1], F32, tag="eps128")
        nc.gpsimd.memset(eps128[:], EPS)

        ag_in = dram.tile([LOCAL], BF)
        ag_out = dram.tile([4 * LOCAL], BF)

        oT = [oT_pool.tile([128, ROWS], BF, tag="oT", name=f"oT{i}") for i in range(PAIRS)]

        # ============ phase A: qkv projections + AllGather of k/v ============
        with (
            tc.tile_pool(name="xt", bufs=1) as xt_pool,
            tc.tile_pool(name="qkT", bufs=12) as qkT_pool,
            tc.tile_pool(name="vsb", bufs=4) as vsb_pool,
            tc.tile_pool(name="wv", bufs=1) as wv_pool,
            tc.tile_pool(name="wstream", bufs=4) as ws_pool,
        ):
            xt = xt_pool.tile([128, KT_N, ROWS], BF, tag="xt")
            nc.sync.dma_start(xt[:], xT.rearrange("(g p) q -> p g q", p=128))

            wv_t = wv_pool.tile([128, KT_N, 768], BF, tag="wv")
            for k in range(KT_N):
                nc.sync.dma_start(wv_t[:, k, :], wqkv[k * 128:(k + 1) * 128, 1536:2304])

            qkT = []
            with (
                tc.tile_pool(name="ps_qk", bufs=3, space="PSUM") as ps_qk,
                tc.tile_pool(name="ps_v", bufs=2, space="PSUM") as ps_v,
            ):
                # q/k feature-major: qkT[ci] = [128 feats, 512 rows]
                for ci in range(12):
                    wt = ws_pool.tile([128, KT_N, 128], BF, tag="wqk")
                    nc.sync.dma_start(wt[:], wqkv_r[:, :, ci * 128:(ci + 1) * 128])
                    ps = ps_qk.tile([128, ROWS], F32, tag="psqk")
                    for k in range(KT_N):
                        nc.tensor.matmul(ps[:], wt[:, k, :], xt[:, k, :],
                                         start=(k == 0), stop=(k == KT_N - 1))
                    t = qkT_pool.tile([128, ROWS], BF, tag="qkT", name=f"qkT{ci}")
                    nc.vector.tensor_scalar_add(t[:], ps[:], bqk_t[:, ci:ci + 1])
                    qkT.append(t)
                    if ci >= 6:  # local k chunk -> AG input
                        g = ci - 6
                        nc.sync.dma_start(
                            ag_in[g * 65536:(g + 1) * 65536].rearrange("(p q) -> p q", q=ROWS),
                            t[:],
                        )

                # v row-major with per-head ones column: v_sb[m] = [128 rows, 780]
                for m in range(4):
                    pv = ps_v.tile([128, 768], F32, tag="psv")
                    for lo, hi in ((0, 512), (512, 768)):
                        for k in range(KT_N):
                            nc.tensor.matmul(pv[:, lo:hi], xt[:, k, m * 128:(m + 1) * 128],
                                             wv_t[:, k, lo:hi],
                                             start=(k == 0), stop=(k == KT_N - 1))
                    vt = vsb_pool.tile([128, VW], BF, tag="vsb", name=f"vsb{m}")
                    vv = vt[:].rearrange("p (h c) -> p h c", c=HD + 1)
                    nc.vector.tensor_tensor(
                        vv[:, :, 0:HD],
                        pv[:].rearrange("p (h c) -> p h c", c=HD),
                        bv_b[:].rearrange("p (h c) -> p h c", c=HD),
                        op=AluOpType.add,
                    )
                    nc.gpsimd.memset(vv[:, :, HD:HD + 1], 1.0)
                    nc.sync.dma_start(
                        ag_in[KSZ + m * (128 * VW):KSZ + (m + 1) * (128 * VW)]
                        .rearrange("(p q) -> p q", q=VW),
                        vt[:],
                    )

            nc.gpsimd.collective_compute(
                "AllGather",
                AluOpType.bypass,
                replica_groups=[[0, 1, 2, 3], [4, 5, 6, 7]],
                ins=[ag_in.opt()],
                outs=[ag_out.opt()],
            )

            # ============ phase B: attention ============
            with (
                tc.tile_pool(name="kt", bufs=PAIRS) as kt_pool,
                tc.tile_pool(name="vfull", bufs=JT) as vf_pool,
                tc.tile_pool(name="exp", bufs=6) as exp_pool,
                tc.tile_pool(name="small", bufs=3) as sm_pool,
                tc.tile_pool(name="ps_sc", bufs=2, space="PSUM") as ps_sc,
                tc.tile_pool(name="ps_o", bufs=4, space="PSUM") as ps_o,
            ):
                KTt = []
                for g in range(PAIRS):
                    t = kt_pool.tile([128, T], BF, tag="kt", name=f"kt{g}")
                    for r in range(4):
                        off = r * LOCAL + g * 65536
                        nc.sync.dma_start(
                            t[:, r * ROWS:(r + 1) * ROWS],
                            ag_out[off:off + 65536].rearrange("(p q) -> p q", q=ROWS),
                        )
                    KTt.append(t)
                Vt = []
                for j in range(JT):
                    r, m = j // 4, j % 4
                    off = r * LOCAL + KSZ + m * (128 * VW)
                    t = vf_pool.tile([128, VW], BF, tag="vfull", name=f"vfull{j}")
                    nc.sync.dma_start(
                        t[:], ag_out[off:off + 128 * VW].rearrange("(p q) -> p q", q=VW)
                    )
                    Vt.append(t)

                for hp in range(PAIRS):
                    oA = ps_o.tile([HD + 1, ROWS], F32, tag="oacc")
                    oB = ps_o.tile([HD + 1, ROWS], F32, tag="oacc")
                    for j in range(JT):
                        sA = ps_sc.tile([128, ROWS], F32, tag="scA")
                        sB = ps_sc.tile([128, ROWS], F32, tag="scB")
                        nc.tensor.matmul(sA[:], KTt[hp][0:64, j * 128:(j + 1) * 128],
                                         qkT[hp][0:64, :], start=True, stop=True)
                        nc.tensor.matmul(sB[:], KTt[hp][64:128, j * 128:(j + 1) * 128],
                                         qkT[hp][64:128, :], start=True, stop=True)
                        eA = exp_pool.tile([128, ROWS], BF, tag="exp", name=f"expA_{hp}_{j}")
                        eB = exp_pool.tile([128, ROWS], BF, tag="exp", name=f"expB_{hp}_{j}")
                        nc.scalar.activation(eA[:], sA[:], ACT.Exp, scale=0.125)
                        nc.scalar.activation(eB[:], sB[:], ACT.Exp, scale=0.125)
                        a, b = 2 * hp, 2 * hp + 1
                        nc.tensor.matmul(oA[:], Vt[j][:, a * 65:(a + 1) * 65], eA[:],
                                         start=(j == 0), stop=(j == JT - 1))
                        nc.tensor.matmul(oB[:], Vt[j][:, b * 65:(b + 1) * 65], eB[:],
                                         start=(j == 0), stop=(j == JT - 1))
                    for o_ps, base in ((oA, 0), (oB, 64)):
                        rcp = sm_pool.tile([1, ROWS], F32, tag="rcp")
                        nc.vector.reciprocal(rcp[:], o_ps[HD:HD + 1, :])
                        rcb = sm_pool.tile([64, ROWS], F32, tag="rcb")
                        nc.gpsimd.partition_broadcast(rcb[:], rcp[:])
                        nc.vector.tensor_tensor(
                            oT[hp][base:base + 64, :], o_ps[0:HD, :], rcb[:],
                            op=AluOpType.mult,
                        )

        # ============ phase C: out-proj + LN1 + FFN + LN2 ============
        with (
            tc.tile_pool(name="aoT", bufs=6) as ao_pool,
            tc.tile_pool(name="sq", bufs=6) as sq_pool,
            tc.tile_pool(name="x1T", bufs=6) as x1_pool,
            tc.tile_pool(name="tmp", bufs=2) as tmp_pool,
            tc.tile_pool(name="stat", bufs=2) as st_pool,
            tc.tile_pool(name="wstream2", bufs=3) as ws2_pool,
        ):
            aoT, sqT = [], []
            with (
                tc.tile_pool(name="ps_ao", bufs=3, space="PSUM") as ps_ao,
                tc.tile_pool(name="ps_stat", bufs=1, space="PSUM") as ps_st,
            ):
                for ci in range(6):
                    wt = ws2_pool.tile([128, KT_N, 128], BF, tag="wout")
                    nc.sync.dma_start(wt[:], wout_r[:, :, ci * 128:(ci + 1) * 128])
                    ps = ps_ao.tile([128, ROWS], F32, tag="psao")
                    for k in range(KT_N):
                        nc.tensor.matmul(ps[:], wt[:, k, :], oT[k][:],
                                         start=(k == 0), stop=(k == KT_N - 1))
                    t = ao_pool.tile([128, ROWS], BF, tag="aoT", name=f"aoT{ci}")
                    nc.vector.tensor_scalar_add(t[:], ps[:], bout_t[:, ci:ci + 1])
                    s = sq_pool.tile([128, ROWS], BF, tag="sq", name=f"sq{ci}")
                    nc.vector.tensor_tensor(s[:], t[:], t[:], op=AluOpType.mult)
                    aoT.append(t)
                    sqT.append(s)

                mu_ps = ps_st.tile([1, ROWS], F32, tag="mups")
                ms_ps = ps_st.tile([1, ROWS], F32, tag="msps")
                for k in range(6):
                    nc.tensor.matmul(mu_ps[:], ones_bf[:], aoT[k][:],
                                     start=(k == 0), stop=(k == 5))
                for k in range(6):
                    nc.tensor.matmul(ms_ps[:], ones_bf[:], sqT[k][:],
                                     start=(k == 0), stop=(k == 5))

                mu = st_pool.tile([1, ROWS], F32, tag="mu")
                ms = st_pool.tile([1, ROWS], F32, tag="ms")
                nc.vector.tensor_scalar_mul(mu[:], mu_ps[:], 1.0 / D)
                nc.vector.tensor_scalar_mul(ms[:], ms_ps[:], 1.0 / D)
                var = st_pool.tile([1, ROWS], F32, tag="var")
                nc.vector.tensor_tensor(var[:], mu[:], mu[:], op=AluOpType.mult)
                nc.vector.tensor_sub(var[:], ms[:], var[:])
                std = st_pool.tile([1, ROWS], F32, tag="std")
                nc.scalar.activation(std[:], var[:], ACT.Sqrt, bias=eps1[:])
                rstd = st_pool.tile([1, ROWS], F32, tag="rstd")
                nc.vector.reciprocal(rstd[:], std[:])
                mu_b = st_pool.tile([128, ROWS], F32, tag="mub")
                rstd_b = st_pool.tile([128, ROWS], F32, tag="rstdb")
                nc.gpsimd.partition_broadcast(mu_b[:], mu[:])
                nc.gpsimd.partition_broadcast(rstd_b[:], rstd[:])

                x1T = []
                for ci in range(6):
                    tp = tmp_pool.tile([128, ROWS], F32, tag="tmp")
                    nc.vector.tensor_sub(tp[:], aoT[ci][:], mu_b[:])
                    tp2 = tmp_pool.tile([128, ROWS], F32, tag="tmp2")
                    nc.vector.scalar_tensor_tensor(
                        tp2[:], tp[:], g1_t[:, ci:ci + 1], rstd_b[:],
                        op0=AluOpType.mult, op1=AluOpType.mult,
                    )
                    t = x1_pool.tile([128, ROWS], BF, tag="x1T", name=f"x1T{ci}")
                    nc.vector.tensor_scalar_add(t[:], tp2[:], bt1_t[:, ci:ci + 1])
                    x1T.append(t)

            # FFN
            with (
                tc.tile_pool(name="hT", bufs=24) as h_pool,
                tc.tile_pool(name="wff2", bufs=24) as wf2_pool,
                tc.tile_pool(name="ln2", bufs=1) as ln_pool,
                tc.tile_pool(name="ps_f1", bufs=3, space="PSUM") as ps_f1,
                tc.tile_pool(name="ps_f2", bufs=2, space="PSUM") as ps_f2,
            ):
                hT = []
                for ci in range(24):
                    wt = ws2_pool.tile([128, KT_N, 128], BF, tag="wff1")
                    nc.sync.dma_start(wt[:], wff1_r[:, :, ci * 128:(ci + 1) * 128])
                    ps = ps_f1.tile([128, ROWS], F32, tag="psf1")
                    for k in range(KT_N):
                        nc.tensor.matmul(ps[:], wt[:, k, :], x1T[k][:],
                                         start=(k == 0), stop=(k == KT_N - 1))
                    t = h_pool.tile([128, ROWS], BF, tag="hT", name=f"hT{ci}")
                    nc.scalar.activation(t[:], ps[:], ACT.Gelu, bias=bff1_t[:, ci:ci + 1])
                    hT.append(t)

                wf2 = []
                for k in range(24):
                    t = wf2_pool.tile([128, 768], BF, tag="wff2", name=f"wff2_{k}")
                    nc.sync.dma_start(t[:], wff2[k * 128:(k + 1) * 128, :])
                    wf2.append(t)

                for m in range(4):
                    pf = ps_f2.tile([128, 768], F32, tag="psf2")
                    for lo, hi in ((0, 512), (512, 768)):
                        for k in range(24):
                            nc.tensor.matmul(pf[:, lo:hi],
                                             hT[k][:, m * 128:(m + 1) * 128],
                                             wf2[k][:, lo:hi],
                                             start=(k == 0), stop=(k == 23))
                    ffn = ln_pool.tile([128, 768], F32, tag="ffn")
                    nc.vector.tensor_tensor(ffn[:], pf[:], bff2_b[:], op=AluOpType.add)
                    s = st_pool.tile([128, 1], F32, tag="s2")
                    nc.vector.reduce_sum(s[:], ffn[:], axis=AX)
                    mu2 = st_pool.tile([128, 1], F32, tag="mu2")
                    nc.vector.tensor_scalar_mul(mu2[:], s[:], 1.0 / D)
                    cen = ln_pool.tile([128, 768], F32, tag="cen")
                    nc.vector.tensor_scalar(cen[:], ffn[:], mu2[:], None,
                                            op0=AluOpType.subtract)
                    sq2 = ln_pool.tile([128, 768], F32, tag="sq2")
                    nc.vector.tensor_tensor(sq2[:], cen[:], cen[:], op=AluOpType.mult)
                    vs = st_pool.tile([128, 1], F32, tag="vs")
                    nc.vector.reduce_sum(vs[:], sq2[:], axis=AX)
                    std2 = st_pool.tile([128, 1], F32, tag="std2")
                    nc.scalar.activation(std2[:], vs[:], ACT.Sqrt, bias=eps128[:], scale=1.0 / D)
                    rstd2 = st_pool.tile([128, 1], F32, tag="rstd2")
                    nc.vector.reciprocal(rstd2[:], std2[:])
                    o1 = ln_pool.tile([128, 768], F32, tag="o1")
                    nc.vector.scalar_tensor_tensor(
                        o1[:], cen[:], rstd2[:], g2_b[:],
                        op0=AluOpType.mult, op1=AluOpType.mult,
                    )
                    o2 = ln_pool.tile([128, 768], F32, tag="o2")
                    nc.vector.tensor_tensor(o2[:], o1[:], bt2_b[:], op=AluOpType.add)
                    nc.sync.dma_start(out[m * 128:(m + 1) * 128, :], o2[:])


_NC = None


def _get_nc():
    global _NC
    if _NC is None:
        nc = bacc.Bacc("TRN2", target_bir_lowering=False, debug=False,
                       num_devices=N_CORES)
        io = {
            "xT": nc.dram_tensor("xT", [D, ROWS], BF, kind="ExternalInput").ap(),
            "w_qkv": nc.dram_tensor("w_qkv", [D, 3 * D], BF, kind="ExternalInput").ap(),
            "w_out": nc.dram_tensor("w_out", [D, D], BF, kind="ExternalInput").ap(),
            "w_ff1": nc.dram_tensor("w_ff1", [D, F], BF, kind="ExternalInput").ap(),
            "w_ff2": nc.dram_tensor("w_ff2", [F, D], BF, kind="ExternalInput").ap(),
            "b_qkv": nc.dram_tensor("b_qkv", [3 * D], F32, kind="ExternalInput").ap(),
            "b_out": nc.dram_tensor("b_out", [D], F32, kind="ExternalInput").ap(),
            "b_ff1": nc.dram_tensor("b_ff1", [F], F32, kind="ExternalInput").ap(),
            "b_ff2": nc.dram_tensor("b_ff2", [D], F32, kind="ExternalInput").ap(),
            "g1": nc.dram_tensor("g1", [D], F32, kind="ExternalInput").ap(),
            "bt1": nc.dram_tensor("bt1", [D], F32, kind="ExternalInput").ap(),
            "g2": nc.dram_tensor("g2", [D], F32, kind="ExternalInput").ap(),
            "bt2": nc.dram_tensor("bt2", [D], F32, kind="ExternalInput").ap(),
            "out": nc.dram_tensor("out", [ROWS, D], F32, kind="ExternalOutput").ap(),
        }
        with tile.TileContext(nc) as tc:
            _body(nc, tc, io)
        nc.compile()
        _NC = nc
    return _NC


def run(inputs: dict, trace: bool = False, trace_kwargs=None, tmpdir=None):
    nc = _get_nc()
    bf = ml_dtypes.bfloat16
    x = np.ascontiguousarray(inputs["x"], dtype=np.float32)
    shared = {
        "w_qkv": np.ascontiguousarray(inputs["w_qkv"], dtype=np.float32).astype(bf),
        "w_out": np.ascontiguousarray(inputs["w_out"], dtype=np.float32).astype(bf),
        "w_ff1": np.ascontiguousarray(inputs["w_ff1"], dtype=np.float32).astype(bf),
        "w_ff2": np.ascontiguousarray(inputs["w_ff2"], dtype=np.float32).astype(bf),
        "b_qkv": np.ascontiguousarray(inputs["b_qkv"], dtype=np.float32),
        "b_out": np.ascontiguousarray(inputs["b_out"], dtype=np.float32),
        "b_ff1": np.ascontiguousarray(inputs["b_ff1"], dtype=np.float32),
        "b_ff2": np.ascontiguousarray(inputs["b_ff2"], dtype=np.float32),
        "g1": np.ascontiguousarray(inputs["g1"], dtype=np.float32),
        "bt1": np.ascontiguousarray(inputs["bt1"], dtype=np.float32),
        "g2": np.ascontiguousarray(inputs["g2"], dtype=np.float32),
        "bt2": np.ascontiguousarray(inputs["bt2"], dtype=np.float32),
    }
    in_maps = []
    for c in range(N_CORES):
        b, m = c // 4, c % 4
        xc = np.ascontiguousarray(x[b, m * ROWS:(m + 1) * ROWS, :].T).astype(bf)
        in_maps.append({"xT": xc, **shared})
    kw = {}
    if trace:
        kw["trace"] = True
        if trace_kwargs:
            kw["trace_kwargs"] = trace_kwargs
    if tmpdir:
        kw["tmpdir"] = tmpdir
    res = bass_utils.run_bass_kernel_spmd(nc, in_maps, core_ids=list(range(N_CORES)), **kw)
    out = np.empty((B, T, D), dtype=np.float32)
    for c in range(N_CORES):
        b, m = c // 4, c % 4
        out[b, m * ROWS:(m + 1) * ROWS, :] = res.results[c]["out"]
    return out, res


def kernel(**inputs) -> np.ndarray:
    out, _ = run(inputs)
    return out


# revision 8
# speedup vs baseline: 1.1285x; 1.1285x over previous
"""Trainium2 Bass kernel for a dense transformer decoder layer (no-residual variant).

Shapes (hardcoded): x [2, 2048, 768], H=12 heads, head_dim=64, FFN dim 3072, fp32 I/O.

Sharding: 8 cores; core c handles batch b=c//4, token rows [512*(c%4), 512*(c%4+1)).
Each core redundantly computes k/v for its FULL batch (2048 rows) from xT_full —
this removes all collectives (a 4-rank AllGather measured ~100us, worse than the
~46us of extra PE work). q is computed only for the core's own 512 rows. The
attn@v matmul carries a per-head ones column in v so it also produces the softmax
denominator. v-compute is interleaved into head-pair 0's attention loop so ACT
(exp) overlaps PE work.

Compute dtype: bf16 matmul operands, fp32 PSUM accumulation and LN/softmax math.
"""

import sys

for p in ("/opt/trn_rl_repo",):
    if p not in sys.path:
        sys.path.insert(0, p)

import numpy as np
import ml_dtypes

import concourse.bass as bass
import concourse.mybir as mybir
import concourse.tile as tile
from concourse import bacc, bass_utils
from concourse.alu_op_type import AluOpType

BF = mybir.dt.bfloat16
F32 = mybir.dt.float32
AX = mybir.AxisListType.X
ACT = mybir.ActivationFunctionType

B, T, D, F, H, HD = 2, 2048, 768, 3072, 12, 64
N_CORES = 8
ROWS = 512          # token rows per core
KT_N = D // 128     # 6 k-tiles over D
PAIRS = H // 2      # 6 head pairs
JT = T // 128       # 16 j-tiles over full sequence
VW = H * (HD + 1)   # 780: v tile width, 65 per head (64 + ones column)
EPS = 1e-5


def _body(nc, tc, io):
    xTf, xTo = io["xT_full"], io["xT_own"]
    wqkv, wout, wff1, wff2 = io["w_qkv"], io["w_out"], io["w_ff1"], io["w_ff2"]
    bqkv, bout, bff1, bff2 = io["b_qkv"], io["b_out"], io["b_ff1"], io["b_ff2"]
    g1, bt1, g2, bt2 = io["g1"], io["bt1"], io["g2"], io["bt2"]
    out = io["out"]

    wqkv_r = wqkv.rearrange("(g p) c -> p g c", p=128)   # [128, 6, 2304]
    wout_r = wout.rearrange("(g p) c -> p g c", p=128)   # [128, 6, 768]
    wff1_r = wff1.rearrange("(g p) c -> p g c", p=128)   # [128, 6, 3072]

    with (
        tc.tile_pool(name="consts", bufs=1) as cpool,
        tc.tile_pool(name="oT", bufs=PAIRS) as oT_pool,
    ):
        # --- constant / bias tiles ---
        bqk_t = cpool.tile([128, 12], F32, tag="bqk")     # per-partition q/k bias
        nc.sync.dma_start(bqk_t[:], bqkv[0:1536].rearrange("(j p) -> p j", p=128))
        bout_t = cpool.tile([128, 6], F32, tag="bout")
        nc.sync.dma_start(bout_t[:], bout.rearrange("(j p) -> p j", p=128))
        bff1_t = cpool.tile([128, 24], F32, tag="bff1")
        nc.sync.dma_start(bff1_t[:], bff1.rearrange("(j p) -> p j", p=128))
        g1_t = cpool.tile([128, 6], F32, tag="g1")
        nc.sync.dma_start(g1_t[:], g1.rearrange("(j p) -> p j", p=128))
        bt1_t = cpool.tile([128, 6], F32, tag="bt1")
        nc.sync.dma_start(bt1_t[:], bt1.rearrange("(j p) -> p j", p=128))

        # free-axis vectors -> broadcast to all 128 partitions
        row = cpool.tile([1, 768 * 3], F32, tag="rowvec")
        nc.sync.dma_start(row[:, 0:768], bff2[None, :])
        nc.sync.dma_start(row[:, 768:1536], g2[None, :])
        nc.sync.dma_start(row[:, 1536:2304], bt2[None, :])
        bv_row = cpool.tile([1, 768], F32, tag="bvrow")
        nc.sync.dma_start(bv_row[:], bqkv[None, 1536:2304])

        bff2_b = cpool.tile([128, 768], F32, tag="bff2b")
        g2_b = cpool.tile([128, 768], F32, tag="g2b")
        bt2_b = cpool.tile([128, 768], F32, tag="bt2b")
        bv_b = cpool.tile([128, 768], F32, tag="bvb")
        nc.gpsimd.partition_broadcast(bff2_b[:], row[:, 0:768])
        nc.gpsimd.partition_broadcast(g2_b[:], row[:, 768:1536])
        nc.gpsimd.partition_broadcast(bt2_b[:], row[:, 1536:2304])
        nc.gpsimd.partition_broadcast(bv_b[:], bv_row[:])

        ones_bf = cpool.tile([128, 1], BF, tag="ones")
        nc.gpsimd.memset(ones_bf[:], 1.0)
        eps1 = cpool.tile([1, 1], F32, tag="eps1")
        nc.gpsimd.memset(eps1[:], EPS)
        eps128 = cpool.tile([128, 1], F32, tag="eps128")
        nc.gpsimd.memset(eps128[:], EPS)

        oT = [oT_pool.tile([128, ROWS], BF, tag="oT", name=f"oT{i}") for i in range(PAIRS)]

        # ============ phases A+B: qkv + attention (PSUM pools coexist) ============
        with (
            tc.tile_pool(name="xtf", bufs=1) as xtf_pool,
            tc.tile_pool(name="xto", bufs=1) as xto_pool,
            tc.tile_pool(name="qkT", bufs=6) as qkT_pool,
            tc.tile_pool(name="kt", bufs=PAIRS) as kt_pool,
            tc.tile_pool(name="vfull", bufs=JT) as vf_pool,
            tc.tile_pool(name="wv", bufs=1) as wv_pool,
            tc.tile_pool(name="wstream", bufs=3) as ws_pool,
            tc.tile_pool(name="exp", bufs=3) as exp_pool,
            tc.tile_pool(name="small", bufs=3) as sm_pool,
            tc.tile_pool(name="ps_mm", bufs=2, space="PSUM") as ps_mm,
            tc.tile_pool(name="ps_sc", bufs=2, space="PSUM") as ps_sc,
            tc.tile_pool(name="ps_o", bufs=2, space="PSUM") as ps_o,
        ):
            xtf = xtf_pool.tile([128, KT_N, T], BF, tag="xtf")
            nc.sync.dma_start(xtf[:], xTf.rearrange("(g p) q -> p g q", p=128))
            xto = xto_pool.tile([128, KT_N, ROWS], BF, tag="xto")
            nc.sync.dma_start(xto[:], xTo.rearrange("(g p) q -> p g q", p=128))

            wv_t = wv_pool.tile([128, KT_N, 768], BF, tag="wv")
            for k in range(KT_N):
                nc.sync.dma_start(wv_t[:, k, :], wqkv[k * 128:(k + 1) * 128, 1536:2304])

            # k feature-major over FULL batch: KT[pair] = [128 feats, 2048 rows]
            KTt = [kt_pool.tile([128, T], BF, tag="kt", name=f"kt{g}") for g in range(PAIRS)]
            for ci in range(6):
                wt = ws_pool.tile([128, KT_N, 128], BF, tag="wqk", name=f"wk{ci}")
                nc.sync.dma_start(wt[:], wqkv_r[:, :, 768 + ci * 128:768 + (ci + 1) * 128])
                for n in range(4):
                    ps = ps_mm.tile([128, ROWS], F32, tag="psmm", name=f"psk{ci}_{n}")
                    for k in range(KT_N):
                        nc.tensor.matmul(ps[:], wt[:, k, :], xtf[:, k, n * ROWS:(n + 1) * ROWS],
                                         start=(k == 0), stop=(k == KT_N - 1))
                    nc.vector.tensor_scalar_add(KTt[ci][:, n * ROWS:(n + 1) * ROWS], ps[:],
                                                bqk_t[:, 6 + ci:7 + ci])

            # q feature-major for own rows
            qkT = []
            for ci in range(6):
                wt = ws_pool.tile([128, KT_N, 128], BF, tag="wqk", name=f"wq{ci}")
                nc.sync.dma_start(wt[:], wqkv_r[:, :, ci * 128:(ci + 1) * 128])
                ps = ps_mm.tile([128, ROWS], F32, tag="psmm", name=f"psq{ci}")
                for k in range(KT_N):
                    nc.tensor.matmul(ps[:], wt[:, k, :], xto[:, k, :],
                                     start=(k == 0), stop=(k == KT_N - 1))
                t = qkT_pool.tile([128, ROWS], BF, tag="qkT", name=f"qkT{ci}")
                nc.vector.tensor_scalar_add(t[:], ps[:], bqk_t[:, ci:ci + 1])
                qkT.append(t)

            Vt = [vf_pool.tile([128, VW], BF, tag="vfull", name=f"vfull{j}") for j in range(JT)]

            def compute_v(j):
                # v row-major over full batch, per-head ones column
                vv = Vt[j][:].rearrange("p (h c) -> p h c", c=HD + 1)
                for lo, hi in ((0, 512), (512, 768)):
                    pv = ps_mm.tile([128, hi - lo], F32, tag="psmm", name=f"psv{j}_{lo}")
                    for k in range(KT_N):
                        nc.tensor.matmul(pv[:], xtf[:, k, j * 128:(j + 1) * 128],
                                         wv_t[:, k, lo:hi],
                                         start=(k == 0), stop=(k == KT_N - 1))
                    nh = (hi - lo) // HD
                    h0 = lo // HD
                    nc.vector.tensor_tensor(
                        vv[:, h0:h0 + nh, 0:HD],
                        pv[:].rearrange("p (h c) -> p h c", c=HD),
                        bv_b[:, lo:hi].rearrange("p (h c) -> p h c", c=HD),
                        op=AluOpType.add,
                    )
                nc.gpsimd.memset(vv[:, :, HD:HD + 1], 1.0)

            for hp in range(PAIRS):
                ha, hb = 2 * hp, 2 * hp + 1
                oA = ps_o.tile([HD + 1, ROWS], F32, tag="oacc", name=f"oA{hp}")
                oB = ps_o.tile([HD + 1, ROWS], F32, tag="oacc", name=f"oB{hp}")

                def flush(pend):
                    e, j = pend
                    nc.tensor.matmul(oA[:], Vt[j][:, ha * 65:(ha + 1) * 65], e[:, 0:512],
                                     start=(j == 0), stop=(j == JT - 1))
                    nc.tensor.matmul(oB[:], Vt[j][:, hb * 65:(hb + 1) * 65], e[:, 512:1024],
                                     start=(j == 0), stop=(j == JT - 1))

                pend = None  # (expAB, j) waiting for its attn@v matmuls
                for j in range(JT):
                    if hp == 0:
                        compute_v(j)
                    sAB = ps_sc.tile([128, 1024], F32, tag="scAB", name=f"s{hp}_{j}")
                    nc.tensor.matmul(sAB[:, 0:512], KTt[hp][0:64, j * 128:(j + 1) * 128],
                                     qkT[hp][0:64, :], start=True, stop=True)
                    nc.tensor.matmul(sAB[:, 512:1024], KTt[hp][64:128, j * 128:(j + 1) * 128],
                                     qkT[hp][64:128, :], start=True, stop=True)
                    e = exp_pool.tile([128, 1024], BF, tag="exp", name=f"exp{hp}_{j}")
                    nc.scalar.activation(e[:], sAB[:], ACT.Exp, scale=0.125)
                    if pend is not None:
                        flush(pend)
                    pend = (e, j)
                flush(pend)

                for o_ps, base in ((oA, 0), (oB, 64)):
                    rcp = sm_pool.tile([1, ROWS], F32, tag="rcp")
                    nc.vector.reciprocal(rcp[:], o_ps[HD:HD + 1, :])
                    rcb = sm_pool.tile([64, ROWS], F32, tag="rcb")
                    nc.gpsimd.partition_broadcast(rcb[:], rcp[:])
                    nc.vector.tensor_tensor(
                        oT[hp][base:base + 64, :], o_ps[0:HD, :], rcb[:],
                        op=AluOpType.mult,
                    )

        # ============ phase C: out-proj + LN1 + FFN + LN2 ============
        with (
            tc.tile_pool(name="aoT", bufs=6) as ao_pool,
            tc.tile_pool(name="sq", bufs=6) as sq_pool,
            tc.tile_pool(name="x1T", bufs=6) as x1_pool,
            tc.tile_pool(name="tmp", bufs=2) as tmp_pool,
            tc.tile_pool(name="stat", bufs=2) as st_pool,
            tc.tile_pool(name="wstream2", bufs=3) as ws2_pool,
        ):
            aoT, sqT = [], []
            with (
                tc.tile_pool(name="ps_ao", bufs=3, space="PSUM") as ps_ao,
                tc.tile_pool(name="ps_stat", bufs=1, space="PSUM") as ps_st,
            ):
                for ci in range(6):
                    wt = ws2_pool.tile([128, KT_N, 128], BF, tag="wout")
                    nc.sync.dma_start(wt[:], wout_r[:, :, ci * 128:(ci + 1) * 128])
                    ps = ps_ao.tile([128, ROWS], F32, tag="psao")
                    for k in range(KT_N):
                        nc.tensor.matmul(ps[:], wt[:, k, :], oT[k][:],
                                         start=(k == 0), stop=(k == KT_N - 1))
                    t = ao_pool.tile([128, ROWS], BF, tag="aoT", name=f"aoT{ci}")
                    nc.vector.tensor_scalar_add(t[:], ps[:], bout_t[:, ci:ci + 1])
                    s = sq_pool.tile([128, ROWS], BF, tag="sq", name=f"sq{ci}")
                    nc.vector.tensor_tensor(s[:], t[:], t[:], op=AluOpType.mult)
                    aoT.append(t)
                    sqT.append(s)

                mu_ps = ps_st.tile([1, ROWS], F32, tag="mups")
                ms_ps = ps_st.tile([1, ROWS], F32, tag="msps")
                for k in range(6):
                    nc.tensor.matmul(mu_ps[:], ones_bf[:], aoT[k][:],
                                     start=(k == 0), stop=(k == 5))
                for k in range(6):
                    nc.tensor.matmul(ms_ps[:], ones_bf[:], sqT[k][:],
                                     start=(k == 0), stop=(k == 5))

                mu = st_pool.tile([1, ROWS], F32, tag="mu")
                ms = st_pool.tile([1, ROWS], F32, tag="ms")
                nc.vector.tensor_scalar_mul(mu[:], mu_ps[:], 1.0 / D)
                nc.vector.tensor_scalar_mul(ms[:], ms_ps[:], 1.0 / D)
                var = st_pool.tile([1, ROWS], F32, tag="var")
                nc.vector.tensor_tensor(var[:], mu[:], mu[:], op=AluOpType.mult)
                nc.vector.tensor_sub(var[:], ms[:], var[:])
                std = st_pool.tile([1, ROWS], F32, tag="std")
                nc.scalar.activation(std[:], var[:], ACT.Sqrt, bias=eps1[:])
                rstd = st_pool.tile([1, ROWS], F32, tag="rstd")
                nc.vector.reciprocal(rstd[:], std[:])
                mu_b = st_pool.tile([128, ROWS], F32, tag="mub")
                rstd_b = st_pool.tile([128, ROWS], F32, tag="rstdb")
                nc.gpsimd.partition_broadcast(mu_b[:], mu[:])
                nc.gpsimd.partition_broadcast(rstd_b[:], rstd[:])

                x1T = []
                for ci in range(6):
                    tp = tmp_pool.tile([128, ROWS], F32, tag="tmp")
                    nc.vector.tensor_sub(tp[:], aoT[ci][:], mu_b[:])
                    tp2 = tmp_pool.tile([128, ROWS], F32, tag="tmp2")
                    nc.vector.scalar_tensor_tensor(
                        tp2[:], tp[:], g1_t[:, ci:ci + 1], rstd_b[:],
                        op0=AluOpType.mult, op1=AluOpType.mult,
                    )
                    t = x1_pool.tile([128, ROWS], BF, tag="x1T", name=f"x1T{ci}")
                    nc.vector.tensor_scalar_add(t[:], tp2[:], bt1_t[:, ci:ci + 1])
                    x1T.append(t)

            # FFN
            with (
                tc.tile_pool(name="hT", bufs=24) as h_pool,
                tc.tile_pool(name="wff2", bufs=24) as wf2_pool,
                tc.tile_pool(name="ln2", bufs=1) as ln_pool,
                tc.tile_pool(name="ps_f1", bufs=3, space="PSUM") as ps_f1,
                tc.tile_pool(name="ps_f2", bufs=2, space="PSUM") as ps_f2,
            ):
                hT = []
                for ci in range(24):
                    wt = ws2_pool.tile([128, KT_N, 128], BF, tag="wff1")
                    nc.sync.dma_start(wt[:], wff1_r[:, :, ci * 128:(ci + 1) * 128])
                    ps = ps_f1.tile([128, ROWS], F32, tag="psf1")
                    for k in range(KT_N):
                        nc.tensor.matmul(ps[:], wt[:, k, :], x1T[k][:],
                                         start=(k == 0), stop=(k == KT_N - 1))
                    t = h_pool.tile([128, ROWS], BF, tag="hT", name=f"hT{ci}")
                    nc.scalar.activation(t[:], ps[:], ACT.Gelu, bias=bff1_t[:, ci:ci + 1])
                    hT.append(t)

                wf2 = []
                for k in range(24):
                    t = wf2_pool.tile([128, 768], BF, tag="wff2", name=f"wff2_{k}")
                    nc.sync.dma_start(t[:], wff2[k * 128:(k + 1) * 128, :])
                    wf2.append(t)

                for m in range(4):
                    pf = ps_f2.tile([128, 768], F32, tag="psf2")
                    for lo, hi in ((0, 512), (512, 768)):
                        for k in range(24):
                            nc.tensor.matmul(pf[:, lo:hi],
                                             hT[k][:, m * 128:(m + 1) * 128],
                                             wf2[k][:, lo:hi],
                                             start=(k == 0), stop=(k == 23))
                    ffn = ln_pool.tile([128, 768], F32, tag="ffn")
                    nc.vector.tensor_tensor(ffn[:], pf[:], bff2_b[:], op=AluOpType.add)
                    s = st_pool.tile([128, 1], F32, tag="s2")
                    nc.vector.reduce_sum(s[:], ffn[:], axis=AX)
                    mu2 = st_pool.tile([128, 1], F32, tag="mu2")
                    nc.vector.tensor_scalar_mul(mu2[:], s[:], 1.0 / D)
                    cen = ln_pool.tile([128, 768], F32, tag="cen")
                    nc.vector.tensor_scalar(cen[:], ffn[:], mu2[:], None,
                                            op0=AluOpType.subtract)
                    sq2 = ln_pool.tile([128, 768], F32, tag="sq2")
                    nc.vector.tensor_tensor(sq2[:], cen[:], cen[:], op=AluOpType.mult)
                    vs = st_pool.tile([128, 1], F32, tag="vs")
                    nc.vector.reduce_sum(vs[:], sq2[:], axis=AX)
                    std2 = st_pool.tile([128, 1], F32, tag="std2")
                    nc.scalar.activation(std2[:], vs[:], ACT.Sqrt, bias=eps128[:], scale=1.0 / D)
                    rstd2 = st_pool.tile([128, 1], F32, tag="rstd2")
                    nc.vector.reciprocal(rstd2[:], std2[:])
                    o1 = ln_pool.tile([128, 768], F32, tag="o1")
                    nc.vector.scalar_tensor_tensor(
                        o1[:], cen[:], rstd2[:], g2_b[:],
                        op0=AluOpType.mult, op1=AluOpType.mult,
                    )
                    o2 = ln_pool.tile([128, 768], F32, tag="o2")
                    nc.vector.tensor_tensor(o2[:], o1[:], bt2_b[:], op=AluOpType.add)
                    nc.sync.dma_start(out[m * 128:(m + 1) * 128, :], o2[:])


_NC = None


def _get_nc():
    global _NC
    if _NC is None:
        nc = bacc.Bacc("TRN2", target_bir_lowering=False, debug=False,
                       num_devices=N_CORES)
        io = {
            "xT_full": nc.dram_tensor("xT_full", [D, T], BF, kind="ExternalInput").ap(),
            "xT_own": nc.dram_tensor("xT_own", [D, ROWS], BF, kind="ExternalInput").ap(),
            "w_qkv": nc.dram_tensor("w_qkv", [D, 3 * D], BF, kind="ExternalInput").ap(),
            "w_out": nc.dram_tensor("w_out", [D, D], BF, kind="ExternalInput").ap(),
            "w_ff1": nc.dram_tensor("w_ff1", [D, F], BF, kind="ExternalInput").ap(),
            "w_ff2": nc.dram_tensor("w_ff2", [F, D], BF, kind="ExternalInput").ap(),
            "b_qkv": nc.dram_tensor("b_qkv", [3 * D], F32, kind="ExternalInput").ap(),
            "b_out": nc.dram_tensor("b_out", [D], F32, kind="ExternalInput").ap(),
            "b_ff1": nc.dram_tensor("b_ff1", [F], F32, kind="ExternalInput").ap(),
            "b_ff2": nc.dram_tensor("b_ff2", [D], F32, kind="ExternalInput").ap(),
            "g1": nc.dram_tensor("g1", [D], F32, kind="ExternalInput").ap(),
            "bt1": nc.dram_tensor("bt1", [D], F32, kind="ExternalInput").ap(),
            "g2": nc.dram_tensor("g2", [D], F32, kind="ExternalInput").ap(),
            "bt2": nc.dram_tensor("bt2", [D], F32, kind="ExternalInput").ap(),
            "out": nc.dram_tensor("out", [ROWS, D], F32, kind="ExternalOutput").ap(),
        }
        with tile.TileContext(nc) as tc:
            _body(nc, tc, io)
        nc.compile()
        _NC = nc
    return _NC


def run(inputs: dict, trace: bool = False, trace_kwargs=None, tmpdir=None):
    nc = _get_nc()
    bf = ml_dtypes.bfloat16
    x = np.ascontiguousarray(inputs["x"], dtype=np.float32)
    shared = {
        "w_qkv": np.ascontiguousarray(inputs["w_qkv"], dtype=np.float32).astype(bf),
        "w_out": np.ascontiguousarray(inputs["w_out"], dtype=np.float32).astype(bf),
        "w_ff1": np.ascontiguousarray(inputs["w_ff1"], dtype=np.float32).astype(bf),
        "w_ff2": np.ascontiguousarray(inputs["w_ff2"], dtype=np.float32).astype(bf),
        "b_qkv": np.ascontiguousarray(inputs["b_qkv"], dtype=np.float32),
        "b_out": np.ascontiguousarray(inputs["b_out"], dtype=np.float32),
        "b_ff1": np.ascontiguousarray(inputs["b_ff1"], dtype=np.float32),
        "b_ff2": np.ascontiguousarray(inputs["b_ff2"], dtype=np.float32),
        "g1": np.ascontiguousarray(inputs["g1"], dtype=np.float32),
        "bt1": np.ascontiguousarray(inputs["bt1"], dtype=np.float32),
        "g2": np.ascontiguousarray(inputs["g2"], dtype=np.float32),
        "bt2": np.ascontiguousarray(inputs["bt2"], dtype=np.float32),
    }
    xT_b = [np.ascontiguousarray(x[b].T).astype(bf) for b in range(B)]
    in_maps = []
    for c in range(N_CORES):
        b, m = c // 4, c % 4
        in_maps.append({
            "xT_full": xT_b[b],
            "xT_own": np.ascontiguousarray(xT_b[b][:, m * ROWS:(m + 1) * ROWS]),
            **shared,
        })
    kw = {}
    if trace:
        kw["trace"] = True
        if trace_kwargs:
            kw["trace_kwargs"] = trace_kwargs
    if tmpdir:
        kw["tmpdir"] = tmpdir
    res = bass_utils.run_bass_kernel_spmd(nc, in_maps, core_ids=list(range(N_CORES)), **kw)
    out = np.empty((B, T, D), dtype=np.float32)
    for c in range(N_CORES):
        b, m = c // 4, c % 4
        out[b, m * ROWS:(m + 1) * ROWS, :] = res.results[c]["out"]
    return out, res


def kernel(**inputs) -> np.ndarray:
    out, _ = run(inputs)
    return out


# revision 11
# speedup vs baseline: 1.2479x; 1.1058x over previous
"""Trainium2 Bass kernel for a dense transformer decoder layer (no-residual variant).

Shapes (hardcoded): x [2, 2048, 768], H=12 heads, head_dim=64, FFN dim 3072, fp32 I/O.

Sharding: 8 cores; core c handles batch b=c//4, token rows [512*(c%4), 512*(c%4+1)).
Each core redundantly computes k/v for its FULL batch (2048 rows) — this removes
all collectives (a 4-rank AllGather measured ~100us, worse than the ~46us of extra
PE work). q is computed only for the core's own 512 rows. v carries a per-head
ones column so the attn@v matmul also produces the softmax denominator; softmax
normalization is deferred off the PSUM-slot critical path (fast psum->sbuf copies,
batched reciprocal overlapped with the next head-pair). Host pre-packs x into tile
layout and biases into two tensors so every DMA is a large contiguous transfer
(DMA issue on the sync engine is serial, ~0.65us each).

Compute dtype: bf16 matmul operands, fp32 PSUM accumulation and LN/softmax math.
"""

import sys

for p in ("/opt/trn_rl_repo",):
    if p not in sys.path:
        sys.path.insert(0, p)

import numpy as np
import ml_dtypes

import concourse.bass as bass
import concourse.mybir as mybir
import concourse.tile as tile
from concourse import bacc, bass_utils
from concourse.alu_op_type import AluOpType

BF = mybir.dt.bfloat16
F32 = mybir.dt.float32
AX = mybir.AxisListType.X
ACT = mybir.ActivationFunctionType

B, T, D, F, H, HD = 2, 2048, 768, 3072, 12, 64
N_CORES = 8
ROWS = 512          # token rows per core
KT_N = D // 128     # 6 k-tiles over D
PAIRS = H // 2      # 6 head pairs
JT = T // 128       # 16 j-tiles over full sequence
VW = H * (HD + 1)   # 780: v tile width, 65 per head (64 + ones column)
EPS = 1e-5


def _body(nc, tc, io):
    xTf, xTo = io["xT_full"], io["xT_own"]
    wqkv, wout, wff1, wff2 = io["w_qkv"], io["w_out"], io["w_ff1"], io["w_ff2"]
    cpk, rowv = io["cpk"], io["rowv"]
    out = io["out"]

    with (
        tc.tile_pool(name="consts", bufs=1) as cpool,
        tc.tile_pool(name="oT", bufs=PAIRS) as oT_pool,
        tc.tile_pool(name="x1T", bufs=6) as x1_pool,
    ):
        # --- packed constants ---
        # cpk: [128, 54] f32 = bqk(12) | bout(6) | bff1(24) | g1(6) | bt1(6)
        cpk_t = cpool.tile([128, 54], F32, tag="cpk")
        nc.sync.dma_start(cpk_t[:], cpk[:])
        bqk_t = cpk_t[:, 0:12]
        bout_t = cpk_t[:, 12:18]
        bff1_t = cpk_t[:, 18:42]
        g1_t = cpk_t[:, 42:48]
        bt1_t = cpk_t[:, 48:54]

        # rowv: [1, 3072] f32 = bff2 | g2 | bt2 | b_v  (each 768, free axis)
        row = cpool.tile([1, 3072], F32, tag="rowvec")
        nc.sync.dma_start(row[:], rowv[:])
        bff2_b = cpool.tile([128, 768], F32, tag="bff2b")
        g2_b = cpool.tile([128, 768], F32, tag="g2b")
        bt2_b = cpool.tile([128, 768], F32, tag="bt2b")
        bv_b = cpool.tile([128, 768], F32, tag="bvb")
        nc.gpsimd.partition_broadcast(bff2_b[:], row[:, 0:768])
        nc.gpsimd.partition_broadcast(g2_b[:], row[:, 768:1536])
        nc.gpsimd.partition_broadcast(bt2_b[:], row[:, 1536:2304])
        nc.gpsimd.partition_broadcast(bv_b[:], row[:, 2304:3072])

        ones_bf = cpool.tile([128, 1], BF, tag="ones")
        nc.gpsimd.memset(ones_bf[:], 1.0)
        eps1 = cpool.tile([1, 1], F32, tag="eps1")
        nc.gpsimd.memset(eps1[:], EPS)
        eps128 = cpool.tile([128, 1], F32, tag="eps128")
        nc.gpsimd.memset(eps128[:], EPS)

        oT = [oT_pool.tile([128, ROWS], BF, tag="oT", name=f"oT{i}") for i in range(PAIRS)]

        # ============ phases A+B: qkv + attention ============
        with (
            tc.tile_pool(name="xtf", bufs=1) as xtf_pool,
            tc.tile_pool(name="xto", bufs=1) as xto_pool,
            tc.tile_pool(name="wqkv", bufs=1) as wq_pool,
            tc.tile_pool(name="qkT", bufs=6) as qkT_pool,
            tc.tile_pool(name="kt", bufs=PAIRS) as kt_pool,
            tc.tile_pool(name="vfull", bufs=JT) as vf_pool,
            tc.tile_pool(name="exp", bufs=3) as exp_pool,
            tc.tile_pool(name="osb", bufs=12) as osb_pool,
            tc.tile_pool(name="small", bufs=2) as sm_pool,
            tc.tile_pool(name="ps_mm", bufs=2, space="PSUM") as ps_mm,
            tc.tile_pool(name="ps_sc", bufs=2, space="PSUM") as ps_sc,
            tc.tile_pool(name="ps_o", bufs=2, space="PSUM") as ps_o,
        ):
            # x in tile layout, host pre-packed: [128, 6, T] contiguous
            xto = xto_pool.tile([128, KT_N, ROWS], BF, tag="xto")
            nc.sync.dma_start(xto[:], xTo[:])
            # full w_qkv rows: [128, 6, 2304]; q cols 0:768, k 768:1536, v 1536:2304
            wqkv_sb = wq_pool.tile([128, KT_N, 3 * D], BF, tag="wqkv")
            nc.sync.dma_start(wqkv_sb[:], wqkv.rearrange("(g p) c -> p g c", p=128))
            xtf = xtf_pool.tile([128, KT_N, T], BF, tag="xtf")
            nc.sync.dma_start(xtf[:], xTf[:])

            # q feature-major for own rows (starts earliest: needs only xto)
            qkT = []
            for ci in range(6):
                ps = ps_mm.tile([128, ROWS], F32, tag="psmm", name=f"psq{ci}")
                for k in range(KT_N):
                    nc.tensor.matmul(ps[:], wqkv_sb[:, k, ci * 128:(ci + 1) * 128],
                                     xto[:, k, :],
                                     start=(k == 0), stop=(k == KT_N - 1))
                t = qkT_pool.tile([128, ROWS], BF, tag="qkT", name=f"qkT{ci}")
                nc.vector.tensor_scalar_add(t[:], ps[:], bqk_t[:, ci:ci + 1])
                qkT.append(t)

            # k feature-major over FULL batch: KT[pair] = [128 feats, 2048 rows]
            KTt = [kt_pool.tile([128, T], BF, tag="kt", name=f"kt{g}") for g in range(PAIRS)]
            for ci in range(6):
                for n in range(4):
                    ps = ps_mm.tile([128, ROWS], F32, tag="psmm", name=f"psk{ci}_{n}")
                    for k in range(KT_N):
                        nc.tensor.matmul(ps[:], wqkv_sb[:, k, 768 + ci * 128:768 + (ci + 1) * 128],
                                         xtf[:, k, n * ROWS:(n + 1) * ROWS],
                                         start=(k == 0), stop=(k == KT_N - 1))
                    nc.vector.tensor_scalar_add(KTt[ci][:, n * ROWS:(n + 1) * ROWS], ps[:],
                                                bqk_t[:, 6 + ci:7 + ci])

            Vt = [vf_pool.tile([128, VW], BF, tag="vfull", name=f"vfull{j}") for j in range(JT)]

            def compute_v(j):
                # v row-major over full batch, per-head ones column
                vv = Vt[j][:].rearrange("p (h c) -> p h c", c=HD + 1)
                for lo, hi in ((0, 512), (512, 768)):
                    pv = ps_mm.tile([128, hi - lo], F32, tag="psmm", name=f"psv{j}_{lo}")
                    for k in range(KT_N):
                        nc.tensor.matmul(pv[:], xtf[:, k, j * 128:(j + 1) * 128],
                                         wqkv_sb[:, k, 1536 + lo:1536 + hi],
                                         start=(k == 0), stop=(k == KT_N - 1))
                    nh = (hi - lo) // HD
                    h0 = lo // HD
                    nc.vector.tensor_tensor(
                        vv[:, h0:h0 + nh, 0:HD],
                        pv[:].rearrange("p (h c) -> p h c", c=HD),
                        bv_b[:, lo:hi].rearrange("p (h c) -> p h c", c=HD),
                        op=AluOpType.add,
                    )
                nc.gpsimd.memset(vv[:, :, HD:HD + 1], 1.0)

            # per-head raw attention outputs, denominator in row 64 (psum evacuated fast)
            osb = [osb_pool.tile([HD + 1, ROWS], F32, tag="osb", name=f"osb{h}") for h in range(H)]

            def normalize(hp):
                # overlapped with the next pair's attention (DVE/GPS only)
                for i, base in ((0, 0), (1, 64)):
                    h = 2 * hp + i
                    rec = sm_pool.tile([1, ROWS], F32, tag="rec", name=f"rec{h}")
                    nc.vector.reciprocal(rec[:], osb[h][HD:HD + 1, :])
                    rcb = sm_pool.tile([64, ROWS], F32, tag="rcb", name=f"rcb{h}")
                    nc.gpsimd.partition_broadcast(rcb[:], rec[:])
                    nc.vector.tensor_tensor(
                        oT[hp][base:base + 64, :], osb[h][0:HD, :], rcb[:],
                        op=AluOpType.mult,
                    )

            for hp in range(PAIRS):
                ha, hb = 2 * hp, 2 * hp + 1
                oA = ps_o.tile([HD + 1, ROWS], F32, tag="oacc", name=f"oA{hp}")
                oB = ps_o.tile([HD + 1, ROWS], F32, tag="oacc", name=f"oB{hp}")

                def flush(pend):
                    e, j = pend
                    nc.tensor.matmul(oA[:], Vt[j][:, ha * 65:(ha + 1) * 65], e[:, 0:512],
                                     start=(j == 0), stop=(j == JT - 1))
                    nc.tensor.matmul(oB[:], Vt[j][:, hb * 65:(hb + 1) * 65], e[:, 512:1024],
                                     start=(j == 0), stop=(j == JT - 1))

                pend = None  # (expAB, j) waiting for its attn@v matmuls
                for j in range(JT):
                    if hp == 0:
                        compute_v(j)
                    sAB = ps_sc.tile([128, 1024], F32, tag="scAB", name=f"s{hp}_{j}")
                    nc.tensor.matmul(sAB[:, 0:512], KTt[hp][0:64, j * 128:(j + 1) * 128],
                                     qkT[hp][0:64, :], start=True, stop=True)
                    nc.tensor.matmul(sAB[:, 512:1024], KTt[hp][64:128, j * 128:(j + 1) * 128],
                                     qkT[hp][64:128, :], start=True, stop=True)
                    e = exp_pool.tile([128, 1024], BF, tag="exp", name=f"exp{hp}_{j}")
                    nc.scalar.activation(e[:], sAB[:], ACT.Exp, scale=0.125)
                    if pend is not None:
                        flush(pend)
                    pend = (e, j)
                flush(pend)

                # fast psum evacuation (frees the oacc slots in ~1us)
                for o_ps, h in ((oA, ha), (oB, hb)):
                    nc.vector.tensor_copy(osb[h][:], o_ps[:])
                if hp > 0:
                    normalize(hp - 1)
            normalize(PAIRS - 1)

        # ============ phase C: out-proj + LN1 ============
        with (
            tc.tile_pool(name="aoT", bufs=6) as ao_pool,
            tc.tile_pool(name="sq", bufs=6) as sq_pool,
            tc.tile_pool(name="tmp", bufs=2) as tmp_pool,
            tc.tile_pool(name="stat", bufs=1) as st_pool,
            tc.tile_pool(name="wout", bufs=1) as wo_pool,
            tc.tile_pool(name="ps_ao", bufs=3, space="PSUM") as ps_ao,
            tc.tile_pool(name="ps_stat", bufs=1, space="PSUM") as ps_st,
        ):
            wout_sb = wo_pool.tile([128, KT_N, D], BF, tag="wout")
            nc.sync.dma_start(wout_sb[:], wout.rearrange("(g p) c -> p g c", p=128))

            aoT, sqT = [], []
            for ci in range(6):
                ps = ps_ao.tile([128, ROWS], F32, tag="psao")
                for k in range(KT_N):
                    nc.tensor.matmul(ps[:], wout_sb[:, k, ci * 128:(ci + 1) * 128], oT[k][:],
                                     start=(k == 0), stop=(k == KT_N - 1))
                t = ao_pool.tile([128, ROWS], BF, tag="aoT", name=f"aoT{ci}")
                nc.vector.tensor_scalar_add(t[:], ps[:], bout_t[:, ci:ci + 1])
                s = sq_pool.tile([128, ROWS], BF, tag="sq", name=f"sq{ci}")
                nc.vector.tensor_tensor(s[:], t[:], t[:], op=AluOpType.mult)
                aoT.append(t)
                sqT.append(s)

            mu_ps = ps_st.tile([1, ROWS], F32, tag="mups")
            ms_ps = ps_st.tile([1, ROWS], F32, tag="msps")
            for k in range(6):
                nc.tensor.matmul(mu_ps[:], ones_bf[:], aoT[k][:],
                                 start=(k == 0), stop=(k == 5))
            for k in range(6):
                nc.tensor.matmul(ms_ps[:], ones_bf[:], sqT[k][:],
                                 start=(k == 0), stop=(k == 5))

            mu = st_pool.tile([1, ROWS], F32, tag="mu")
            ms = st_pool.tile([1, ROWS], F32, tag="ms")
            nc.vector.tensor_scalar_mul(mu[:], mu_ps[:], 1.0 / D)
            nc.vector.tensor_scalar_mul(ms[:], ms_ps[:], 1.0 / D)
            var = st_pool.tile([1, ROWS], F32, tag="var")
            nc.vector.tensor_tensor(var[:], mu[:], mu[:], op=AluOpType.mult)
            nc.vector.tensor_sub(var[:], ms[:], var[:])
            std = st_pool.tile([1, ROWS], F32, tag="std")
            nc.scalar.activation(std[:], var[:], ACT.Sqrt, bias=eps1[:])
            rstd = st_pool.tile([1, ROWS], F32, tag="rstd")
            nc.vector.reciprocal(rstd[:], std[:])
            mu_b = st_pool.tile([128, ROWS], F32, tag="mub")
            rstd_b = st_pool.tile([128, ROWS], F32, tag="rstdb")
            nc.gpsimd.partition_broadcast(mu_b[:], mu[:])
            nc.gpsimd.partition_broadcast(rstd_b[:], rstd[:])

            x1T = []
            for ci in range(6):
                tp = tmp_pool.tile([128, ROWS], F32, tag="tmp")
                nc.vector.tensor_sub(tp[:], aoT[ci][:], mu_b[:])
                tp2 = tmp_pool.tile([128, ROWS], F32, tag="tmp2")
                nc.vector.scalar_tensor_tensor(
                    tp2[:], tp[:], g1_t[:, ci:ci + 1], rstd_b[:],
                    op0=AluOpType.mult, op1=AluOpType.mult,
                )
                t = x1_pool.tile([128, ROWS], BF, tag="x1T", name=f"x1T{ci}")
                nc.vector.tensor_scalar_add(t[:], tp2[:], bt1_t[:, ci:ci + 1])
                x1T.append(t)

        # ============ phase D: FFN + LN2 ============
        with (
            tc.tile_pool(name="wff1", bufs=1) as wf1_pool,
            tc.tile_pool(name="wff2p", bufs=1) as wf2_pool,
            tc.tile_pool(name="hT", bufs=24) as h_pool,
            tc.tile_pool(name="ln2", bufs=1) as ln_pool,
            tc.tile_pool(name="lnstat", bufs=2) as ls_pool,
            tc.tile_pool(name="ps_f1", bufs=3, space="PSUM") as ps_f1,
            tc.tile_pool(name="ps_f2", bufs=2, space="PSUM") as ps_f2,
        ):
            wff1_sb = wf1_pool.tile([128, KT_N, F], BF, tag="wff1")
            nc.sync.dma_start(wff1_sb[:], wff1.rearrange("(g p) c -> p g c", p=128))
            wff2_sb = wf2_pool.tile([128, 24, D], BF, tag="wff2")
            nc.sync.dma_start(wff2_sb[:], wff2.rearrange("(g p) c -> p g c", p=128))

            hT = []
            for ci in range(24):
                ps = ps_f1.tile([128, ROWS], F32, tag="psf1")
                for k in range(KT_N):
                    nc.tensor.matmul(ps[:], wff1_sb[:, k, ci * 128:(ci + 1) * 128], x1T[k][:],
                                     start=(k == 0), stop=(k == KT_N - 1))
                t = h_pool.tile([128, ROWS], BF, tag="hT", name=f"hT{ci}")
                nc.scalar.activation(t[:], ps[:], ACT.Gelu, bias=bff1_t[:, ci:ci + 1])
                hT.append(t)

            for m in range(4):
                pf = ps_f2.tile([128, 768], F32, tag="psf2")
                for lo, hi in ((0, 512), (512, 768)):
                    for k in range(24):
                        nc.tensor.matmul(pf[:, lo:hi],
                                         hT[k][:, m * 128:(m + 1) * 128],
                                         wff2_sb[:, k, lo:hi],
                                         start=(k == 0), stop=(k == 23))
                ffn = ln_pool.tile([128, 768], F32, tag="ffn")
                nc.vector.tensor_tensor(ffn[:], pf[:], bff2_b[:], op=AluOpType.add)
                s = ls_pool.tile([128, 1], F32, tag="s2")
                nc.vector.reduce_sum(s[:], ffn[:], axis=AX)
                mu2 = ls_pool.tile([128, 1], F32, tag="mu2")
                nc.vector.tensor_scalar_mul(mu2[:], s[:], 1.0 / D)
                cen = ln_pool.tile([128, 768], F32, tag="cen")
                nc.vector.tensor_scalar(cen[:], ffn[:], mu2[:], None,
                                        op0=AluOpType.subtract)
                sq2 = ln_pool.tile([128, 768], F32, tag="sq2")
                nc.vector.tensor_tensor(sq2[:], cen[:], cen[:], op=AluOpType.mult)
                vs = ls_pool.tile([128, 1], F32, tag="vs")
                nc.vector.reduce_sum(vs[:], sq2[:], axis=AX)
                std2 = ls_pool.tile([128, 1], F32, tag="std2")
                nc.scalar.activation(std2[:], vs[:], ACT.Sqrt, bias=eps128[:], scale=1.0 / D)
                rstd2 = ls_pool.tile([128, 1], F32, tag="rstd2")
                nc.vector.reciprocal(rstd2[:], std2[:])
                o1 = ln_pool.tile([128, 768], F32, tag="o1")
                nc.vector.scalar_tensor_tensor(
                    o1[:], cen[:], rstd2[:], g2_b[:],
                    op0=AluOpType.mult, op1=AluOpType.mult,
                )
                o2 = ln_pool.tile([128, 768], F32, tag="o2")
                nc.vector.tensor_tensor(o2[:], o1[:], bt2_b[:], op=AluOpType.add)
                nc.sync.dma_start(out[m * 128:(m + 1) * 128, :], o2[:])


_NC = None


def _get_nc():
    global _NC
    if _NC is None:
        nc = bacc.Bacc("TRN2", target_bir_lowering=False, debug=False,
                       num_devices=N_CORES)
        io = {
            "xT_full": nc.dram_tensor("xT_full", [128, KT_N, T], BF, kind="ExternalInput").ap(),
            "xT_own": nc.dram_tensor("xT_own", [128, KT_N, ROWS], BF, kind="ExternalInput").ap(),
            "w_qkv": nc.dram_tensor("w_qkv", [D, 3 * D], BF, kind="ExternalInput").ap(),
            "w_out": nc.dram_tensor("w_out", [D, D], BF, kind="ExternalInput").ap(),
            "w_ff1": nc.dram_tensor("w_ff1", [D, F], BF, kind="ExternalInput").ap(),
            "w_ff2": nc.dram_tensor("w_ff2", [F, D], BF, kind="ExternalInput").ap(),
            "cpk": nc.dram_tensor("cpk", [128, 54], F32, kind="ExternalInput").ap(),
            "rowv": nc.dram_tensor("rowv", [1, 3072], F32, kind="ExternalInput").ap(),
            "out": nc.dram_tensor("out", [ROWS, D], F32, kind="ExternalOutput").ap(),
        }
        with tile.TileContext(nc) as tc:
            _body(nc, tc, io)
        nc.compile()
        _NC = nc
    return _NC


def run(inputs: dict, trace: bool = False, trace_kwargs=None, tmpdir=None):
    nc = _get_nc()
    bf = ml_dtypes.bfloat16
    f32 = np.float32
    x = np.ascontiguousarray(inputs["x"], dtype=f32)

    # packed per-feature constants [128, 54]: bqk | bout | bff1 | g1 | bt1
    def col128(v):  # [n*128] -> [128, n]
        v = np.asarray(v, dtype=f32)
        return v.reshape(-1, 128).T

    cpk = np.concatenate(
        [
            col128(np.asarray(inputs["b_qkv"], f32)[0:1536]),
            col128(inputs["b_out"]),
            col128(inputs["b_ff1"]),
            col128(inputs["g1"]),
            col128(inputs["bt1"]),
        ],
        axis=1,
    )
    rowv = np.concatenate(
        [
            np.asarray(inputs["b_ff2"], f32),
            np.asarray(inputs["g2"], f32),
            np.asarray(inputs["bt2"], f32),
            np.asarray(inputs["b_qkv"], f32)[1536:2304],
        ]
    ).reshape(1, 3072)

    shared = {
        "w_qkv": np.ascontiguousarray(inputs["w_qkv"], dtype=f32).astype(bf),
        "w_out": np.ascontiguousarray(inputs["w_out"], dtype=f32).astype(bf),
        "w_ff1": np.ascontiguousarray(inputs["w_ff1"], dtype=f32).astype(bf),
        "w_ff2": np.ascontiguousarray(inputs["w_ff2"], dtype=f32).astype(bf),
        "cpk": np.ascontiguousarray(cpk),
        "rowv": np.ascontiguousarray(rowv),
    }
    # x in tile layout: [D, T] -> [128, 6, T] (partition, k-tile, token)
    xT_b = []
    for b in range(B):
        xt = x[b].T.astype(bf)                       # [768, 2048]
        xT_b.append(np.ascontiguousarray(xt.reshape(KT_N, 128, T).transpose(1, 0, 2)))
    in_maps = []
    for c in range(N_CORES):
        b, m = c // 4, c % 4
        in_maps.append({
            "xT_full": xT_b[b],
            "xT_own": np.ascontiguousarray(xT_b[b][:, :, m * ROWS:(m + 1) * ROWS]),
            **shared,
        })
    kw = {}
    if trace:
        kw["trace"] = True
        if trace_kwargs:
            kw["trace_kwargs"] = trace_kwargs
    if tmpdir:
        kw["tmpdir"] = tmpdir
    res = bass_utils.run_bass_kernel_spmd(nc, in_maps, core_ids=list(range(N_CORES)), **kw)
    out = np.empty((B, T, D), dtype=np.float32)
    for c in range(N_CORES):
        b, m = c // 4, c % 4
        out[b, m * ROWS:(m + 1) * ROWS, :] = res.results[c]["out"]
    return out, res


def kernel(**inputs) -> np.ndarray:
    out, _ = run(inputs)
    return out
